# revision 10
# baseline (speedup 1.0000x reference)
"""Chebyshev approximation kernel for Trainium2 (8 NeuronCores, SPMD data-parallel).

Math: reference computes
    q   = (1-t) * y[:, idx] + t * y[:, idx+1]     # [n_obs, deg]  (interp at nodes)
    out = (q @ basis).reshape(-1)                 # basis = DCT-II-like matrix

Factorization used here (~4x less PE work than the fused K=2048 GEMM):
  1. q' = y @ W'            W' = interp matrix with node columns permuted
                            [nodes 0..511, nodes 1023..512]; banded, 2 nnz/col.
  2. DCT-II radix split: u = q'[:, :512] + q'[:, 512:], v = first - second
     out[:, 2i]   = (u @ De)[:, i]
     out[:, 2i+1] = (v @ Do)[:, i]     De/Do [512, 512] dense (host-built, f64)
  So the big GEMM has K=512 twice (vs K=2048 once) and step 1 is banded.

Device schedule per 512-row superblock, all matmul dtypes bf16:
  - y arrives TRANSPOSED via two xbar DMA-transposes per superblock (bf16
    2-byte dtype; out[p,k,r] = y[r, 128k+p]): no PE transposes, no PSUM
    transpose drains. Grid col 2048 rides an overlap tile (cols 1921..2048)
    whose interp weights are zero except partition 127.
  - step A: 23 node-stationary banded matmuls (lhsT = W' 128x128 bf16 tiles,
    moving = yT k-tile [128, 512] -> LDWEIGHTS fully hidden). PSUM tile per
    node-tile g; left-half tiles (g<4) finish by k=7 and are parked in SBUF
    (ACT copy) so the butterfly has only one PSUM operand.
  - butterfly: DVE tensor_tensor add/sub (psum right-half, sbuf left-half)
    -> u,v in SBUF bf16: drain + butterfly + cast fused.
  - step B: per 128-row block, 2 accumulation groups x 4 matmuls (K=512,
    N=512) from u/v slices vs De/Do; even/odd coeffs interleaved during the
    PSUM drain (DVE/ACT alternate). Output bf16, upcast on host.
  B' of superblock s is emitted inside s+1 to cover the butterfly latency.

Sharding: y rows split 8192/core across 8 cores; constants replicated.
"""

import os
import numpy as np
import ml_dtypes

DEG = 1024
N_OBS = 65536
M_P1 = 2049
N_CORES = 8
ROWS_PER_CORE = N_OBS // N_CORES  # 8192
KT = 17                           # grid k-tiles: 16 full + overlap tile (col 2048)
RB = 128                          # rows per block
SBROWS = 512                      # rows per superblock
NB = SBROWS // RB                 # blocks per superblock

_COMPILED = {}
LAST_RESULTS = None


def _build_mats(x: np.ndarray):
    """Host prep: banded interp weight tiles (permuted node order) + DCT-split
    matrices, all f64 -> bf16. Returns (wt [128,P,128], dmat [128,8,512],
    pairs [(k,g)...] sorted)."""
    x = np.asarray(x, dtype=np.float32)
    k = np.arange(DEG, dtype=np.float32)
    ang = (np.float32(np.pi) * (k + np.float32(0.5))) / np.float32(DEG)
    nodes = np.sort(np.cos(ang.astype(np.float32)).astype(np.float32))
    norm = (np.float32(2.0) - (k == 0).astype(np.float32)).astype(np.float64) / float(
        DEG
    )
    idx = np.clip(np.searchsorted(x, nodes, side="right") - 1, 0, M_P1 - 2)
    a = x[idx]
    b = x[idx + 1]
    t = ((nodes - a) / (b - a)).astype(np.float64)

    # permuted node-column order: [0..511, 1023..512]
    perm = np.concatenate([np.arange(512), np.arange(1023, 511, -1)])
    idxp = idx[perm].astype(np.int64)
    tp = t[perm]

    # nnz of W' [2049, 1024]: (row idxp, 1-tp), (row idxp+1, tp)
    rows_ = np.concatenate([idxp, idxp + 1])
    ws = np.concatenate([1.0 - tp, tp])
    cols_ = np.concatenate([np.arange(DEG), np.arange(DEG)])
    # grid row -> (ktile, partition); row 2048 lives in overlap tile 16
    # (tile 16 = grid cols 1921..2048, so col 2048 -> partition 127)
    ktile = np.where(rows_ == 2048, 16, rows_ // 128)
    part = np.where(rows_ == 2048, 127, rows_ % 128)
    g = cols_ // 128

    pairs = sorted(set(zip(ktile.tolist(), g.tolist())))
    pidx = {pg: i for i, pg in enumerate(pairs)}
    wt = np.zeros((128, len(pairs), 128), dtype=np.float64)
    pvec = np.array([pidx[(kk, gg)] for kk, gg in zip(ktile.tolist(), g.tolist())])
    np.add.at(wt, (part, pvec, cols_ % 128), ws)

    n_ = np.arange(512, dtype=np.float64)
    i_ = np.arange(512, dtype=np.float64)
    De = np.cos(np.pi * np.outer(n_ + 0.5, i_) / 512.0) * norm[0::2][None, :]
    Do = -np.cos(np.pi * np.outer(n_ + 0.5, 2.0 * i_ + 1.0) / 1024.0) * norm[1::2][
        None, :
    ]
    dmat = np.zeros((128, 8, 512), dtype=np.float64)
    for gg in range(4):
        dmat[:, gg, :] = De[128 * gg : 128 * (gg + 1), :]
        dmat[:, 4 + gg, :] = Do[128 * gg : 128 * (gg + 1), :]

    bf = ml_dtypes.bfloat16
    return (
        np.ascontiguousarray(wt.astype(np.float32).astype(bf)),
        np.ascontiguousarray(dmat.astype(np.float32).astype(bf)),
        tuple(pairs),
    )


def build_cheb_kernel(tc, y_ap, wt_ap, d_ap, o_ap, rows, pairs):
    import concourse.mybir as mybir

    nc = tc.nc
    bf = mybir.dt.bfloat16
    f32 = mybir.dt.float32
    SB = rows // SBROWS
    P = len(pairs)
    pidx = {pg: i for i, pg in enumerate(pairs)}
    # per node-tile g first/last pair: each g is its own psum accumulation
    # group and its own psum tile (max ~2 live at a time in k order)
    first_of_g = {}
    last_of_g = {}
    for kk, gg in pairs:
        first_of_g.setdefault(gg, (kk, gg))
        last_of_g[gg] = (kk, gg)

    add_op = mybir.AluOpType.add
    sub_op = mybir.AluOpType.subtract

    with (
        tc.tile_pool(name="consts", bufs=1) as consts,
        tc.tile_pool(name="ytpool", bufs=2) as ytpool,
        tc.tile_pool(name="uvpool", bufs=2) as uvpool,
        tc.tile_pool(name="h0pool", bufs=2) as h0pool,
        tc.tile_pool(name="opool", bufs=4) as opool,
        tc.tile_pool(name="pnp", bufs=3, space="PSUM") as pnp,
        tc.tile_pool(name="pop", bufs=4, space="PSUM") as pop,
    ):
        wt_sb = consts.tile([128, P, 128], bf)
        nc.scalar.dma_start(out=wt_sb, in_=wt_ap)
        d_sb = consts.tile([128, 8, 512], bf)
        d_r = d_ap.rearrange("p (a b) -> p a b", a=8)
        for half in range(2):
            eng = nc.sync if half == 0 else nc.scalar
            eng.dma_start(
                out=d_sb[:, 4 * half : 4 * half + 4, :],
                in_=d_r[:, 4 * half : 4 * half + 4, :],
            )

        ytbs = {}

        def load_yt(s):
            ti = ytpool.tile([128, KT, SBROWS], bf, name="ytb", tag="ytb")
            r0 = s * SBROWS
            # transposed load: ti[p, k, r] = y[r0+r, 128k+p]
            nc.sync.dma_start_transpose(
                ti[:, 0:16, :], y_ap[r0 : r0 + SBROWS, 0:2048]
            )
            nc.sync.dma_start_transpose(
                ti[:, 16, :], y_ap[r0 : r0 + SBROWS, 1921:2049]
            )
            ytbs[s] = ti

        def emit_b(uv, s):
            # step B for superblock s: NB blocks x (even, odd) accumulation groups
            for rb in range(NB):
                osb = opool.tile([128, 512, 2], bf, name="osb", tag="osb")
                for parity in range(2):
                    po = pop.tile([128, 512], f32, name="po", tag="po")
                    for gg in range(4):
                        nc.tensor.matmul(
                            po,
                            uv[:, 4 * parity + gg, rb * 128 : (rb + 1) * 128],
                            d_sb[:, 4 * parity + gg, :],
                            start=(gg == 0),
                            stop=(gg == 3),
                        )
                    if parity == 0:
                        nc.vector.tensor_copy(osb[:, :, 0], po)
                    else:
                        nc.scalar.copy(osb[:, :, 1], po)
                r0 = s * SBROWS + rb * RB
                nc.scalar.dma_start(
                    out=o_ap[r0 : r0 + RB, :], in_=osb.rearrange("p a b -> p (a b)")
                )

        load_yt(0)
        prev = None
        for s in range(SB):
            if s + 1 < SB:
                load_yt(s + 1)
            ytb = ytbs[s]
            uv = uvpool.tile([128, 8, SBROWS], bf, name="uv", tag="uv")
            h0 = h0pool.tile([128, 4, SBROWS], f32, name="h0", tag="h0")
            pn_of_g = {}
            nemit = 0
            for kk, gg in pairs:
                if (kk, gg) == first_of_g[gg]:
                    pn_of_g[gg] = pnp.tile(
                        [128, SBROWS], f32, name=f"pn{gg}", tag="pn"
                    )
                nc.tensor.matmul(
                    pn_of_g[gg],
                    wt_sb[:, pidx[(kk, gg)], :],
                    ytb[:, kk, :],
                    start=((kk, gg) == first_of_g[gg]),
                    stop=((kk, gg) == last_of_g[gg]),
                )
                nemit += 1
                if (kk, gg) == last_of_g[gg]:
                    if gg < 4:
                        # left half done early: park in SBUF so the butterfly
                        # has only one PSUM operand
                        nc.scalar.copy(h0[:, gg, :], pn_of_g[gg])
                    else:
                        b = gg - 4
                        nc.vector.tensor_tensor(
                            uv[:, b, :], pn_of_g[gg], h0[:, b, :], add_op
                        )
                        nc.vector.tensor_tensor(
                            uv[:, 4 + b, :], h0[:, b, :], pn_of_g[gg], sub_op
                        )
                # fill the butterfly window of superblock s with B' of s-1
                if nemit == 14 and prev is not None:
                    emit_b(*prev)
                    del ytbs[prev[1]]
                    prev = None
            if prev is not None:
                emit_b(*prev)
                del ytbs[prev[1]]
            prev = (uv, s)
        emit_b(*prev)


def _build_nc(rows, pairs):
    import concourse.mybir as mybir
    import concourse.tile as tile
    from concourse import bacc

    bf = mybir.dt.bfloat16
    P = len(pairs)
    nc = bacc.Bacc(
        "TRN2",
        target_bir_lowering=False,
        debug=False,
        enable_asserts=False,
        num_devices=N_CORES,
    )
    y_ap = nc.dram_tensor("y", [rows, M_P1], bf, kind="ExternalInput").ap()
    wt_ap = nc.dram_tensor("wt", [128, P * 128], bf, kind="ExternalInput").ap()
    d_ap = nc.dram_tensor("dmat", [128, 8 * 512], bf, kind="ExternalInput").ap()
    o_ap = nc.dram_tensor("o", [rows, DEG], bf, kind="ExternalOutput").ap()
    wt_r = wt_ap.rearrange("p (a b) -> p a b", a=P)
    with tile.TileContext(nc) as tc:
        build_cheb_kernel(tc, y_ap, wt_r, d_ap, o_ap, rows, pairs)
    nc.compile()
    return nc


def _get_compiled(rows, pairs):
    key = (rows, pairs)
    if key not in _COMPILED:
        _COMPILED[key] = _build_nc(rows, pairs)
    return _COMPILED[key]


def kernel(x: np.ndarray, y: np.ndarray) -> np.ndarray:
    global LAST_RESULTS
    from concourse import bass_utils

    bfd = ml_dtypes.bfloat16
    x = np.asarray(x, dtype=np.float32)
    y = np.asarray(y, dtype=np.float32)
    assert y.shape == (N_OBS, M_P1), y.shape
    yb = np.ascontiguousarray(y.astype(bfd))
    wt, dmat, pairs = _build_mats(x)

    nc = _get_compiled(ROWS_PER_CORE, pairs)
    wt2 = wt.reshape(128, -1)
    d2 = dmat.reshape(128, -1)
    in_maps = [
        {
            "y": yb[i * ROWS_PER_CORE : (i + 1) * ROWS_PER_CORE],
            "wt": wt2,
            "dmat": d2,
        }
        for i in range(N_CORES)
    ]
    trace = bool(int(os.environ.get("CHEB_TRACE", "0")))
    res = bass_utils.run_bass_kernel_spmd(
        nc, in_maps, core_ids=list(range(N_CORES)), trace=trace
    )
    LAST_RESULTS = res
    out = np.concatenate(
        [np.asarray(res.results[i]["o"]) for i in range(N_CORES)], axis=0
    )
    return out.astype(np.float32).reshape(-1)


# revision 11
# speedup vs baseline: 1.0442x; 1.0442x over previous
"""Chebyshev approximation kernel for Trainium2 (8 NeuronCores, SPMD data-parallel).

Math: reference computes
    q   = (1-t) * y[:, idx] + t * y[:, idx+1]     # [n_obs, deg]  (interp at nodes)
    out = (q @ basis).reshape(-1)                 # basis = DCT-II-like matrix

Factorization used here (~4x less PE work than the fused K=2048 GEMM):
  1. q' = y @ W'            W' = interp matrix with node columns permuted
                            [nodes 0..511, nodes 1023..512]; banded, 2 nnz/col.
  2. DCT-II radix split: u = q'[:, :512] + q'[:, 512:], v = first - second
     out[:, 2i]   = (u @ De)[:, i]
     out[:, 2i+1] = (v @ Do)[:, i]     De/Do [512, 512] dense (host-built, f64)
  So the big GEMM has K=512 twice (vs K=2048 once) and step 1 is banded.

Device schedule per 512-row superblock, all matmul dtypes bf16:
  - y reaches SBUF transposed (ytb[p,k,r] = y[r, 128k+p]) via a hybrid:
    grid k-tiles 0..KSPLIT-1 go through PE transposes (plain bf16 load +
    128x128 transpose matmuls + DVE/ACT PSUM drains); tiles KSPLIT..16 ride
    the xbar DMA-transpose. The split keeps the PE warm (HAM) and under-
    subscribes every engine instead of over-serializing one.
  - step A: 23 node-stationary banded matmuls (lhsT = W' 128x128 bf16 tiles,
    moving = yT k-tile [128, 512] -> LDWEIGHTS hidden). PSUM tile per node
    tile g; left-half tiles (g<4) finish by k=7 and are parked in SBUF (ACT
    copy) so the butterfly has only one PSUM operand.
  - butterfly: DVE tensor_tensor add/sub (psum right-half + sbuf left-half)
    -> u,v in SBUF bf16 (drain + butterfly + cast fused).
  - step B: per 128-row block, 2 accumulation groups x 4 matmuls (K=512,
    N=512) from u/v slices vs De/Do; even/odd coeffs interleave during the
    PSUM drain (DVE/ACT alternate). Output bf16, upcast on host.
  B' of superblock s-1 is threaded between s's transpose groups to cover
  butterfly latency and drain waits.

Grid col 2048 rides an overlap tile (cols 1921..2048) whose interp weights
are zero except partition 127. Sharding: y rows split 8192/core across 8
cores; constants replicated.
"""

import os
import numpy as np
import ml_dtypes

DEG = 1024
N_OBS = 65536
M_P1 = 2049
N_CORES = 8
ROWS_PER_CORE = N_OBS // N_CORES  # 8192
KT = 17                           # grid k-tiles: 16 full + overlap tile (col 2048)
RB = 128                          # rows per block
SBROWS = 512                      # rows per superblock
NB = SBROWS // RB                 # blocks per superblock
KSPLIT = int(os.environ.get("CHEB_KSPLIT", "8"))  # k < KSPLIT on PE, rest on xbar

_COMPILED = {}
LAST_RESULTS = None


def _build_mats(x: np.ndarray):
    """Host prep: banded interp weight tiles (permuted node order) + DCT-split
    matrices, all f64 -> bf16. Returns (wt [128,P,128], dmat [128,8,512],
    pairs [(k,g)...] sorted)."""
    x = np.asarray(x, dtype=np.float32)
    k = np.arange(DEG, dtype=np.float32)
    ang = (np.float32(np.pi) * (k + np.float32(0.5))) / np.float32(DEG)
    nodes = np.sort(np.cos(ang.astype(np.float32)).astype(np.float32))
    norm = (np.float32(2.0) - (k == 0).astype(np.float32)).astype(np.float64) / float(
        DEG
    )
    idx = np.clip(np.searchsorted(x, nodes, side="right") - 1, 0, M_P1 - 2)
    a = x[idx]
    b = x[idx + 1]
    t = ((nodes - a) / (b - a)).astype(np.float64)

    # permuted node-column order: [0..511, 1023..512]
    perm = np.concatenate([np.arange(512), np.arange(1023, 511, -1)])
    idxp = idx[perm].astype(np.int64)
    tp = t[perm]

    # nnz of W' [2049, 1024]: (row idxp, 1-tp), (row idxp+1, tp)
    rows_ = np.concatenate([idxp, idxp + 1])
    ws = np.concatenate([1.0 - tp, tp])
    cols_ = np.concatenate([np.arange(DEG), np.arange(DEG)])
    # grid row -> (ktile, partition); row 2048 lives in overlap tile 16
    # (tile 16 = grid cols 1921..2048, so col 2048 -> partition 127)
    ktile = np.where(rows_ == 2048, 16, rows_ // 128)
    part = np.where(rows_ == 2048, 127, rows_ % 128)
    g = cols_ // 128

    pairs = sorted(set(zip(ktile.tolist(), g.tolist())))
    pidx = {pg: i for i, pg in enumerate(pairs)}
    wt = np.zeros((128, len(pairs), 128), dtype=np.float64)
    pvec = np.array([pidx[(kk, gg)] for kk, gg in zip(ktile.tolist(), g.tolist())])
    np.add.at(wt, (part, pvec, cols_ % 128), ws)

    n_ = np.arange(512, dtype=np.float64)
    i_ = np.arange(512, dtype=np.float64)
    De = np.cos(np.pi * np.outer(n_ + 0.5, i_) / 512.0) * norm[0::2][None, :]
    Do = -np.cos(np.pi * np.outer(n_ + 0.5, 2.0 * i_ + 1.0) / 1024.0) * norm[1::2][
        None, :
    ]
    dmat = np.zeros((128, 8, 512), dtype=np.float64)
    for gg in range(4):
        dmat[:, gg, :] = De[128 * gg : 128 * (gg + 1), :]
        dmat[:, 4 + gg, :] = Do[128 * gg : 128 * (gg + 1), :]

    bf = ml_dtypes.bfloat16
    return (
        np.ascontiguousarray(wt.astype(np.float32).astype(bf)),
        np.ascontiguousarray(dmat.astype(np.float32).astype(bf)),
        tuple(pairs),
    )


def build_cheb_kernel(tc, y_ap, wt_ap, d_ap, id_ap, o_ap, rows, pairs):
    import concourse.mybir as mybir

    nc = tc.nc
    bf = mybir.dt.bfloat16
    f32 = mybir.dt.float32
    SB = rows // SBROWS
    P = len(pairs)
    pidx = {pg: i for i, pg in enumerate(pairs)}
    first_of_g = {}
    last_of_g = {}
    for kk, gg in pairs:
        first_of_g.setdefault(gg, (kk, gg))
        last_of_g[gg] = (kk, gg)
    pairs_by_k = {}
    for kk, gg in pairs:
        pairs_by_k.setdefault(kk, []).append(gg)

    add_op = mybir.AluOpType.add
    sub_op = mybir.AluOpType.subtract
    # PE-transpose groups: pairs of k-tiles (one PSUM bank each: [128,2,512] bf16)
    TGROUPS = [(k, min(k + 2, KSPLIT)) for k in range(0, KSPLIT, 2)]
    PCOLS = KSPLIT * 128  # plain-loaded columns

    with (
        tc.tile_pool(name="consts", bufs=1) as consts,
        tc.tile_pool(name="ypool", bufs=2) as ypool,
        tc.tile_pool(name="ytpool", bufs=2) as ytpool,
        tc.tile_pool(name="uvpool", bufs=2) as uvpool,
        tc.tile_pool(name="h0pool", bufs=2) as h0pool,
        tc.tile_pool(name="opool", bufs=4) as opool,
        tc.tile_pool(name="pstp", bufs=2, space="PSUM") as pstp,
        tc.tile_pool(name="pnp", bufs=3, space="PSUM") as pnp,
        tc.tile_pool(name="pop", bufs=3, space="PSUM") as pop,
    ):
        ident = consts.tile([128, 128], bf)
        nc.scalar.dma_start(out=ident, in_=id_ap)
        wt_sb = consts.tile([128, P, 128], bf)
        nc.scalar.dma_start(out=wt_sb, in_=wt_ap)
        d_sb = consts.tile([128, 8, 512], bf)
        d_r = d_ap.rearrange("p (a b) -> p a b", a=8)
        for half in range(2):
            eng = nc.sync if half == 0 else nc.scalar
            eng.dma_start(
                out=d_sb[:, 4 * half : 4 * half + 4, :],
                in_=d_r[:, 4 * half : 4 * half + 4, :],
            )

        yss, ytbs = {}, {}

        def load_y(s):
            r0 = s * SBROWS
            ti = ytpool.tile([128, KT, SBROWS], bf, name="ytb", tag="ytb")
            if KSPLIT < 16:
                # right-half k-tiles transposed by the xbar on the way in
                nc.sync.dma_start_transpose(
                    ti[:, KSPLIT:16, :], y_ap[r0 : r0 + SBROWS, PCOLS:2048]
                )
            nc.sync.dma_start_transpose(
                ti[:, 16, :], y_ap[r0 : r0 + SBROWS, 1921:2049]
            )
            ytbs[s] = ti
            if KSPLIT > 0:
                ys = ypool.tile([128, NB, PCOLS], bf, name="ys", tag="ys")
                src = y_ap[r0 : r0 + SBROWS, 0:PCOLS].rearrange(
                    "(rb p) c -> p rb c", p=128
                )
                nc.sync.dma_start(out=ys, in_=src)
                yss[s] = ys

        def emit_b_block(uv, s, rb):
            osb = opool.tile([128, 512, 2], bf, name="osb", tag="osb")
            for parity in range(2):
                po = pop.tile([128, 512], f32, name="po", tag="po")
                for gg in range(4):
                    nc.tensor.matmul(
                        po,
                        uv[:, 4 * parity + gg, rb * 128 : (rb + 1) * 128],
                        d_sb[:, 4 * parity + gg, :],
                        start=(gg == 0),
                        stop=(gg == 3),
                    )
                if parity == 0:
                    nc.vector.tensor_copy(osb[:, :, 0], po)
                else:
                    nc.scalar.copy(osb[:, :, 1], po)
            r0 = s * SBROWS + rb * RB
            nc.scalar.dma_start(
                out=o_ap[r0 : r0 + RB, :], in_=osb.rearrange("p a b -> p (a b)")
            )

        def emit_a(kk, gg, ytb, uv, h0, pn_of_g):
            if (kk, gg) == first_of_g[gg]:
                pn_of_g[gg] = pnp.tile([128, SBROWS], f32, name=f"pn{gg}", tag="pn")
            nc.tensor.matmul(
                pn_of_g[gg],
                wt_sb[:, pidx[(kk, gg)], :],
                ytb[:, kk, :],
                start=((kk, gg) == first_of_g[gg]),
                stop=((kk, gg) == last_of_g[gg]),
            )
            if (kk, gg) == last_of_g[gg]:
                if gg < 4:
                    # left half done early: park in SBUF so the butterfly
                    # has only one PSUM operand
                    nc.scalar.copy(h0[:, gg, :], pn_of_g[gg])
                else:
                    b = gg - 4
                    nc.vector.tensor_tensor(
                        uv[:, b, :], pn_of_g[gg], h0[:, b, :], add_op
                    )
                    nc.vector.tensor_tensor(
                        uv[:, 4 + b, :], h0[:, b, :], pn_of_g[gg], sub_op
                    )

        load_y(0)
        prev = None
        for s in range(SB):
            if s + 1 < SB:
                load_y(s + 1)
            ytb = ytbs[s]
            uv = uvpool.tile([128, 8, SBROWS], bf, name="uv", tag="uv")
            h0 = h0pool.tile([128, 4, SBROWS], f32, name="h0", tag="h0")
            pn_of_g = {}
            # PE transposes for k < KSPLIT interleaved with step A and B'(s-1)
            for gi, (k0, k1) in enumerate(TGROUPS):
                pst = pstp.tile([128, 2, SBROWS], bf, name="pst", tag="pst")
                for kk in range(k0, k1):
                    for rb in range(NB):
                        nc.tensor.transpose(
                            pst[:, kk - k0, rb * 128 : (rb + 1) * 128],
                            yss[s][:, rb, kk * 128 : (kk + 1) * 128],
                            ident,
                        )
                if gi % 2 == 0:
                    nc.vector.tensor_copy(ytb[:, k0:k1, :], pst[:, 0 : k1 - k0, :])
                else:
                    nc.scalar.copy(ytb[:, k0:k1, :], pst[:, 0 : k1 - k0, :])
                for kk in range(k0, k1):
                    for gg in pairs_by_k.get(kk, []):
                        emit_a(kk, gg, ytb, uv, h0, pn_of_g)
                if prev is not None and gi % 2 == 1:
                    # thread B'(s-1) blocks between transpose groups
                    emit_b_block(prev[0], prev[1], gi // 2)
            # xbar-delivered k-tiles
            for kk in range(KSPLIT, KT):
                for gg in pairs_by_k.get(kk, []):
                    emit_a(kk, gg, ytb, uv, h0, pn_of_g)
            if prev is not None:
                for rb in range(len(TGROUPS) // 2, NB):
                    emit_b_block(prev[0], prev[1], rb)
                del ytbs[prev[1]]
                if prev[1] in yss:
                    del yss[prev[1]]
            prev = (uv, s)
        for rb in range(NB):
            emit_b_block(prev[0], prev[1], rb)


def _build_nc(rows, pairs):
    import concourse.mybir as mybir
    import concourse.tile as tile
    from concourse import bacc

    bf = mybir.dt.bfloat16
    P = len(pairs)
    nc = bacc.Bacc(
        "TRN2",
        target_bir_lowering=False,
        debug=False,
        enable_asserts=False,
        num_devices=N_CORES,
    )
    y_ap = nc.dram_tensor("y", [rows, M_P1], bf, kind="ExternalInput").ap()
    wt_ap = nc.dram_tensor("wt", [128, P * 128], bf, kind="ExternalInput").ap()
    d_ap = nc.dram_tensor("dmat", [128, 8 * 512], bf, kind="ExternalInput").ap()
    id_ap = nc.dram_tensor("ident", [128, 128], bf, kind="ExternalInput").ap()
    o_ap = nc.dram_tensor("o", [rows, DEG], bf, kind="ExternalOutput").ap()
    wt_r = wt_ap.rearrange("p (a b) -> p a b", a=P)
    with tile.TileContext(nc) as tc:
        build_cheb_kernel(tc, y_ap, wt_r, d_ap, id_ap, o_ap, rows, pairs)
    nc.compile()
    return nc


def _get_compiled(rows, pairs):
    key = (rows, pairs, KSPLIT)
    if key not in _COMPILED:
        _COMPILED[key] = _build_nc(rows, pairs)
    return _COMPILED[key]


def kernel(x: np.ndarray, y: np.ndarray) -> np.ndarray:
    global LAST_RESULTS
    from concourse import bass_utils

    bfd = ml_dtypes.bfloat16
    x = np.asarray(x, dtype=np.float32)
    y = np.asarray(y, dtype=np.float32)
    assert y.shape == (N_OBS, M_P1), y.shape
    yb = np.ascontiguousarray(y.astype(bfd))
    wt, dmat, pairs = _build_mats(x)

    nc = _get_compiled(ROWS_PER_CORE, pairs)
    ident = np.ascontiguousarray(np.eye(128, dtype=np.float32).astype(bfd))
    wt2 = wt.reshape(128, -1)
    d2 = dmat.reshape(128, -1)
    in_maps = [
        {
            "y": yb[i * ROWS_PER_CORE : (i + 1) * ROWS_PER_CORE],
            "wt": wt2,
            "dmat": d2,
            "ident": ident,
        }
        for i in range(N_CORES)
    ]
    trace = bool(int(os.environ.get("CHEB_TRACE", "0")))
    res = bass_utils.run_bass_kernel_spmd(
        nc, in_maps, core_ids=list(range(N_CORES)), trace=trace
    )
    LAST_RESULTS = res
    out = np.concatenate(
        [np.asarray(res.results[i]["o"]) for i in range(N_CORES)], axis=0
    )
    return out.astype(np.float32).reshape(-1)


# revision 14
# speedup vs baseline: 1.2261x; 1.1741x over previous
"""Chebyshev approximation kernel for Trainium2 (8 NeuronCores, SPMD data-parallel).

Math: reference computes
    q   = (1-t) * y[:, idx] + t * y[:, idx+1]     # [n_obs, deg]  (interp at nodes)
    out = (q @ basis).reshape(-1)                 # basis = DCT-II-like matrix

Factorization used here (~4x less PE work than the fused K=2048 GEMM):
  1. q' = y @ W'            W' = interp matrix with node columns permuted
                            [nodes 0..511, nodes 1023..512]; banded, 2 nnz/col.
  2. DCT-II radix split: u = q'[:, :512] + q'[:, 512:], v = first - second
     out[:, 2i]   = (u @ De)[:, i]
     out[:, 2i+1] = (v @ Do)[:, i]     De/Do [512, 512] dense (host-built, f64)
  So the big GEMM has K=512 twice (vs K=2048 once) and step 1 is banded.

Device schedule per 512-row superblock, all matmul dtypes bf16:
  - y reaches SBUF transposed (ytb[p,k,r] = y[r, 128k+p]) via a hybrid:
    grid k-tiles 0..KSPLIT-1 go through PE transposes (plain bf16 load +
    128x128 transpose matmuls + DVE/ACT PSUM drains); tiles KSPLIT..16 ride
    the xbar DMA-transpose. The split keeps the PE warm (HAM) and under-
    subscribes every engine instead of over-serializing one.
  - step A: 23 node-stationary banded matmuls (lhsT = W' 128x128 bf16 tiles,
    moving = yT k-tile [128, 512] -> LDWEIGHTS hidden). PSUM tile per node
    tile g; left-half tiles (g<4) finish by k=7 and are parked in SBUF (ACT
    copy) so the butterfly has only one PSUM operand.
  - butterfly: DVE tensor_tensor add/sub (psum right-half + sbuf left-half)
    -> u,v in SBUF bf16 (drain + butterfly + cast fused).
  - step B: per 128-row block, 2 accumulation groups x 4 matmuls (K=512,
    N=512) from u/v slices vs De/Do; even/odd coeffs interleave during the
    PSUM drain (DVE/ACT alternate). Output bf16, upcast on host.
  B' of superblock s-1 is threaded between s's transpose groups to cover
  butterfly latency and drain waits.

Grid col 2048 rides an overlap tile (cols 1921..2048) whose interp weights
are zero except partition 127. Sharding: y rows split 8192/core across 8
cores; constants replicated.
"""

import os
import numpy as np
import ml_dtypes

DEG = 1024
N_OBS = 65536
M_P1 = 2049
N_CORES = 8
ROWS_PER_CORE = N_OBS // N_CORES  # 8192
KT = 17                           # grid k-tiles: 16 full + overlap tile (col 2048)
RB = 128                          # rows per block
SBROWS = 512                      # rows per superblock
NB = SBROWS // RB                 # blocks per superblock
KSPLIT = int(os.environ.get("CHEB_KSPLIT", "8"))  # k < KSPLIT on PE, rest on xbar

_COMPILED = {}
LAST_RESULTS = None


def _build_mats(x: np.ndarray):
    """Host prep: banded interp weight tiles (permuted node order) + DCT-split
    matrices, all f64 -> bf16. Returns (wt [128,P,128], dmat [128,8,512],
    pairs [(k,g)...] sorted)."""
    x = np.asarray(x, dtype=np.float32)
    k = np.arange(DEG, dtype=np.float32)
    ang = (np.float32(np.pi) * (k + np.float32(0.5))) / np.float32(DEG)
    nodes = np.sort(np.cos(ang.astype(np.float32)).astype(np.float32))
    norm = (np.float32(2.0) - (k == 0).astype(np.float32)).astype(np.float64) / float(
        DEG
    )
    idx = np.clip(np.searchsorted(x, nodes, side="right") - 1, 0, M_P1 - 2)
    a = x[idx]
    b = x[idx + 1]
    t = ((nodes - a) / (b - a)).astype(np.float64)

    # permuted node-column order: [0..511, 1023..512]
    perm = np.concatenate([np.arange(512), np.arange(1023, 511, -1)])
    idxp = idx[perm].astype(np.int64)
    tp = t[perm]

    # nnz of W' [2049, 1024]: (row idxp, 1-tp), (row idxp+1, tp)
    rows_ = np.concatenate([idxp, idxp + 1])
    ws = np.concatenate([1.0 - tp, tp])
    cols_ = np.concatenate([np.arange(DEG), np.arange(DEG)])
    # grid row -> (ktile, partition); row 2048 lives in overlap tile 16
    # (tile 16 = grid cols 1921..2048, so col 2048 -> partition 127)
    ktile = np.where(rows_ == 2048, 16, rows_ // 128)
    part = np.where(rows_ == 2048, 127, rows_ % 128)
    g = cols_ // 128

    pairs = sorted(set(zip(ktile.tolist(), g.tolist())))
    pidx = {pg: i for i, pg in enumerate(pairs)}
    wt = np.zeros((128, len(pairs), 128), dtype=np.float64)
    pvec = np.array([pidx[(kk, gg)] for kk, gg in zip(ktile.tolist(), g.tolist())])
    np.add.at(wt, (part, pvec, cols_ % 128), ws)

    n_ = np.arange(512, dtype=np.float64)
    i_ = np.arange(512, dtype=np.float64)
    De = np.cos(np.pi * np.outer(n_ + 0.5, i_) / 512.0) * norm[0::2][None, :]
    Do = -np.cos(np.pi * np.outer(n_ + 0.5, 2.0 * i_ + 1.0) / 1024.0) * norm[1::2][
        None, :
    ]
    dmat = np.zeros((128, 8, 512), dtype=np.float64)
    for gg in range(4):
        dmat[:, gg, :] = De[128 * gg : 128 * (gg + 1), :]
        dmat[:, 4 + gg, :] = Do[128 * gg : 128 * (gg + 1), :]

    bf = ml_dtypes.bfloat16
    return (
        np.ascontiguousarray(wt.astype(np.float32).astype(bf)),
        np.ascontiguousarray(dmat.astype(np.float32).astype(bf)),
        tuple(pairs),
    )


def build_cheb_kernel(tc, y_ap, wt_ap, d_ap, id_ap, o_ap, rows, pairs):
    import concourse.mybir as mybir

    nc = tc.nc
    bf = mybir.dt.bfloat16
    f32 = mybir.dt.float32
    SB = rows // SBROWS
    P = len(pairs)
    pidx = {pg: i for i, pg in enumerate(pairs)}
    first_of_g = {}
    last_of_g = {}
    for kk, gg in pairs:
        first_of_g.setdefault(gg, (kk, gg))
        last_of_g[gg] = (kk, gg)
    pairs_by_k = {}
    for kk, gg in pairs:
        pairs_by_k.setdefault(kk, []).append(gg)

    add_op = mybir.AluOpType.add
    sub_op = mybir.AluOpType.subtract
    # PE-transpose groups: pairs of k-tiles (one PSUM bank each: [128,2,512] bf16)
    TGROUPS = [(k, min(k + 2, KSPLIT)) for k in range(0, KSPLIT, 2)]
    PCOLS = M_P1 if KSPLIT == 17 else KSPLIT * 128  # plain-loaded columns

    with (
        tc.tile_pool(name="consts", bufs=1) as consts,
        tc.tile_pool(name="ypool", bufs=2) as ypool,
        tc.tile_pool(name="ytpool", bufs=2) as ytpool,
        tc.tile_pool(name="uvpool", bufs=2) as uvpool,
        tc.tile_pool(name="h0pool", bufs=2) as h0pool,
        tc.tile_pool(name="opool", bufs=4) as opool,
        tc.tile_pool(name="pstp", bufs=2, space="PSUM") as pstp,
        tc.tile_pool(name="pnp", bufs=3, space="PSUM") as pnp,
        tc.tile_pool(name="pop", bufs=3, space="PSUM") as pop,
    ):
        ident = consts.tile([128, 128], bf)
        nc.scalar.dma_start(out=ident, in_=id_ap)
        wt_sb = consts.tile([128, P, 128], bf)
        nc.scalar.dma_start(out=wt_sb, in_=wt_ap)
        d_sb = consts.tile([128, 8, 512], bf)
        d_r = d_ap.rearrange("p (a b) -> p a b", a=8)
        for half in range(2):
            eng = nc.sync if half == 0 else nc.scalar
            eng.dma_start(
                out=d_sb[:, 4 * half : 4 * half + 4, :],
                in_=d_r[:, 4 * half : 4 * half + 4, :],
            )

        yss, ytbs = {}, {}

        def load_y(s):
            r0 = s * SBROWS
            ti = ytpool.tile([128, KT, SBROWS], bf, name="ytb", tag="ytb")
            if KSPLIT < 16:
                # right-half k-tiles transposed by the xbar on the way in
                nc.sync.dma_start_transpose(
                    ti[:, KSPLIT:16, :], y_ap[r0 : r0 + SBROWS, PCOLS:2048]
                )
            if KSPLIT < 17:
                nc.sync.dma_start_transpose(
                    ti[:, 16, :], y_ap[r0 : r0 + SBROWS, 1921:2049]
                )
            ytbs[s] = ti
            if KSPLIT > 0:
                ys = ypool.tile([128, NB, PCOLS], bf, name="ys", tag="ys")
                src = y_ap[r0 : r0 + SBROWS, 0:PCOLS].rearrange(
                    "(rb p) c -> p rb c", p=128
                )
                nc.sync.dma_start(out=ys, in_=src)
                yss[s] = ys

        def emit_b_block(uv, s, rb):
            osb = opool.tile([128, 512, 2], bf, name="osb", tag="osb")
            for parity in range(2):
                po = pop.tile([128, 512], f32, name="po", tag="po")
                for gg in range(4):
                    nc.tensor.matmul(
                        po,
                        uv[:, 4 * parity + gg, rb * 128 : (rb + 1) * 128],
                        d_sb[:, 4 * parity + gg, :],
                        start=(gg == 0),
                        stop=(gg == 3),
                    )
                if parity == 0:
                    nc.vector.tensor_copy(osb[:, :, 0], po)
                else:
                    nc.scalar.copy(osb[:, :, 1], po)
            r0 = s * SBROWS + rb * RB
            nc.scalar.dma_start(
                out=o_ap[r0 : r0 + RB, :], in_=osb.rearrange("p a b -> p (a b)")
            )

        def emit_a(kk, gg, ytb, uv, h0, pn_of_g):
            if (kk, gg) == first_of_g[gg]:
                pn_of_g[gg] = pnp.tile([128, SBROWS], f32, name=f"pn{gg}", tag="pn")
            nc.tensor.matmul(
                pn_of_g[gg],
                wt_sb[:, pidx[(kk, gg)], :],
                ytb[:, kk, :],
                start=((kk, gg) == first_of_g[gg]),
                stop=((kk, gg) == last_of_g[gg]),
            )
            if (kk, gg) == last_of_g[gg]:
                if gg < 4:
                    # left half done early: park in SBUF so the butterfly
                    # has only one PSUM operand
                    nc.scalar.copy(h0[:, gg, :], pn_of_g[gg])
                else:
                    b = gg - 4
                    nc.vector.tensor_tensor(
                        uv[:, b, :], pn_of_g[gg], h0[:, b, :], add_op
                    )
                    nc.vector.tensor_tensor(
                        uv[:, 4 + b, :], h0[:, b, :], pn_of_g[gg], sub_op
                    )

        load_y(0)
        prev = None
        for s in range(SB):
            if s + 1 < SB:
                load_y(s + 1)
            ytb = ytbs[s]
            uv = uvpool.tile([128, 8, SBROWS], bf, name="uv", tag="uv")
            h0 = h0pool.tile([128, 4, SBROWS], f32, name="h0", tag="h0")
            pn_of_g = {}
            # PE transposes for k < KSPLIT interleaved with step A and B'(s-1);
            # the late-k phase (whose butterflies trail into s+1's transpose
            # window) goes last so the DVE never backs up behind B' drains.
            nb_done = 0
            for gi, (k0, k1) in enumerate(TGROUPS):
                pst = pstp.tile([128, 2, SBROWS], bf, name="pst", tag="pst")
                for kk in range(k0, k1):
                    for rb in range(NB):
                        src = (
                            yss[s][:, rb, kk * 128 : (kk + 1) * 128]
                            if kk < 16
                            else yss[s][:, rb, 1921:2049]
                        )
                        nc.tensor.transpose(
                            pst[:, kk - k0, rb * 128 : (rb + 1) * 128], src, ident
                        )
                if gi % 2 == 0:
                    nc.vector.tensor_copy(ytb[:, k0:k1, :], pst[:, 0 : k1 - k0, :])
                else:
                    nc.scalar.copy(ytb[:, k0:k1, :], pst[:, 0 : k1 - k0, :])
                for kk in range(k0, k1):
                    for gg in pairs_by_k.get(kk, []):
                        emit_a(kk, gg, ytb, uv, h0, pn_of_g)
                if prev is not None and gi % 2 == 1 and nb_done < NB:
                    # thread B'(s-1) blocks between transpose groups
                    emit_b_block(prev[0], prev[1], nb_done)
                    nb_done += 1
            # xbar-delivered k-tiles
            for kk in range(KSPLIT, KT):
                for gg in pairs_by_k.get(kk, []):
                    emit_a(kk, gg, ytb, uv, h0, pn_of_g)
                if prev is not None and nb_done < NB and kk >= 10:
                    emit_b_block(prev[0], prev[1], nb_done)
                    nb_done += 1
            if prev is not None:
                for rb in range(nb_done, NB):
                    emit_b_block(prev[0], prev[1], rb)
                del ytbs[prev[1]]
                if prev[1] in yss:
                    del yss[prev[1]]
            prev = (uv, s)
        for rb in range(NB):
            emit_b_block(prev[0], prev[1], rb)


def _build_nc(rows, pairs):
    import concourse.mybir as mybir
    import concourse.tile as tile
    from concourse import bacc

    bf = mybir.dt.bfloat16
    P = len(pairs)
    nc = bacc.Bacc(
        "TRN2",
        target_bir_lowering=False,
        debug=False,
        enable_asserts=False,
        num_devices=N_CORES,
    )
    y_ap = nc.dram_tensor("y", [rows, M_P1], bf, kind="ExternalInput").ap()
    wt_ap = nc.dram_tensor("wt", [128, P * 128], bf, kind="ExternalInput").ap()
    d_ap = nc.dram_tensor("dmat", [128, 8 * 512], bf, kind="ExternalInput").ap()
    id_ap = nc.dram_tensor("ident", [128, 128], bf, kind="ExternalInput").ap()
    o_ap = nc.dram_tensor("o", [rows, DEG], bf, kind="ExternalOutput").ap()
    wt_r = wt_ap.rearrange("p (a b) -> p a b", a=P)
    with tile.TileContext(nc) as tc:
        build_cheb_kernel(tc, y_ap, wt_r, d_ap, id_ap, o_ap, rows, pairs)
    nc.compile()
    return nc


def _get_compiled(rows, pairs):
    key = (rows, pairs, KSPLIT)
    if key not in _COMPILED:
        _COMPILED[key] = _build_nc(rows, pairs)
    return _COMPILED[key]


def kernel(x: np.ndarray, y: np.ndarray) -> np.ndarray:
    global LAST_RESULTS
    from concourse import bass_utils

    bfd = ml_dtypes.bfloat16
    x = np.asarray(x, dtype=np.float32)
    y = np.asarray(y, dtype=np.float32)
    assert y.shape == (N_OBS, M_P1), y.shape
    yb = np.ascontiguousarray(y.astype(bfd))
    wt, dmat, pairs = _build_mats(x)

    nc = _get_compiled(ROWS_PER_CORE, pairs)
    ident = np.ascontiguousarray(np.eye(128, dtype=np.float32).astype(bfd))
    wt2 = wt.reshape(128, -1)
    d2 = dmat.reshape(128, -1)
    in_maps = [
        {
            "y": yb[i * ROWS_PER_CORE : (i + 1) * ROWS_PER_CORE],
            "wt": wt2,
            "dmat": d2,
            "ident": ident,
        }
        for i in range(N_CORES)
    ]
    trace = bool(int(os.environ.get("CHEB_TRACE", "0")))
    res = bass_utils.run_bass_kernel_spmd(
        nc, in_maps, core_ids=list(range(N_CORES)), trace=trace
    )
    LAST_RESULTS = res
    out = np.concatenate(
        [np.asarray(res.results[i]["o"]) for i in range(N_CORES)], axis=0
    )
    return out.astype(np.float32).reshape(-1)
